# revision 5
# baseline (speedup 1.0000x reference)
"""MultiResolutionHashEncoding Trainium2 kernel (v1 redesign).

Per core (batch sharded 8 ways, 262144 elements = 128 partitions x 2048):
  - Tables stored as packed bf16 pairs in int32, one 32768-entry slice per
    partition (slice p%16); all 8 gpsimd-core groups read the same compact
    DRAM copy (replication happens in the 8 HBM->SBUF loads).
  - Whole-level hash on DVE+ACT: floor via ACT round + fused is_lt
    correction, prime multiplies split Khi/Klo to stay f32-exact,
    19-bit arithmetic via fused two-scalar tensor_scalar ops.
  - ap_gather (d=1 int32) fetches candidates on all 16 partitions of each
    group; a second tiny gather produces {-1,0} masks; DVE AND selects.
  - PE sums each 16-partition group via block-diagonal ones (bf16),
    PSUM evacuated to bf16 by ACT (mostly) and DVE (some, for balance).
  - Output bf16 [L, 16, 8, 4096] per core, unscrambled + f32-cast on host.
"""

import numpy as np
import ml_dtypes

import concourse.tile_utils as tile_utils

tile_utils.max_sbuf_usage = 206 * 1024

import concourse.bacc as bacc
import concourse.tile as tile
import concourse.mybir as mybir
from concourse.bass_utils import run_bass_kernel_spmd

AluOp = mybir.AluOpType
ActFn = mybir.ActivationFunctionType
dt = mybir.dt

N_LEVELS = 16
N_FEATS = 2
TABLE_SIZE = 524288  # 2**19
RESOLUTIONS = [16, 23, 32, 45, 64, 91, 128, 181, 256, 362, 512, 724, 1024,
               1448, 2048, 2896]
BATCH = 2_097_152
N_CORES = 8

P = 128
BC = BATCH // N_CORES          # 262144 elements per core
SPP = BC // P                  # 2048 elements per partition
SLICE = 32768                  # table entries per partition slice
MASK19 = 0x7FFFF

K1 = 2654435761 & MASK19       # 489905
K2 = 805459861 & MASK19        # 95765
KH1, KL1 = K1 >> 7, K1 & 127   # 3827, 49
KH2, KL2 = K2 >> 7, K2 & 127   # 748, 21

HC = 512                       # hash sub-chunk columns
NHC = SPP // HC                # 4 hash sub-chunks per level
GC = 1024                      # gather-chunk indices per gpsimd core
NGC = (16 * SPP) // GC         # 32 gather chunks per level
ICH = GC // 16                 # 64 idx columns per partition per chunk
EVAC_DVE_MOD = 8               # chunk c evacs on DVE when c % MOD == MOD-1

LAST_EXEC_SECONDS = None
LAST_RESULT = None
LAST_TIMES = {}


def build_nc():
    nc = bacc.Bacc(None, target_bir_lowering=False)

    coordsd = nc.dram_tensor("coords3", [3, P, SPP], dt.float32,
                             kind="ExternalInput")
    tblr = nc.dram_tensor("tblr", [N_LEVELS, 16, SLICE], dt.int32,
                          kind="ExternalInput")
    b16d = nc.dram_tensor("b16", [P, 8], dt.bfloat16, kind="ExternalInput")
    indd = nc.dram_tensor("ind", [P, 16], dt.int32, kind="ExternalInput")
    outd = nc.dram_tensor("out", [N_LEVELS, NGC // 2, 8, 2 * GC * N_FEATS],
                          dt.bfloat16, kind="ExternalOutput")

    with tile.TileContext(nc) as tc:
        with (
            tc.tile_pool(name="constp", bufs=1) as constp,
            tc.tile_pool(name="tabp", bufs=1) as tabp,
            tc.tile_pool(name="hashio", bufs=2) as hashio,
            tc.tile_pool(name="hashp", bufs=1) as hashp,
            tc.tile_pool(name="lhp", bufs=2) as lhp,
            tc.tile_pool(name="gbufp", bufs=2) as gbufp,
            tc.tile_pool(name="evp", bufs=2) as evp,
            tc.tile_pool(name="psp", bufs=2, space="PSUM") as psp,
        ):
            b16 = constp.tile([P, 8], dt.bfloat16, tag="b16")
            nc.sync.dma_start(b16[:], b16d[:])
            ind = constp.tile([P, 16], dt.int32, tag="ind")
            nc.sync.dma_start(ind[:], indd[:])

            for lvl in range(N_LEVELS):
                R = float(RESOLUTIONS[lvl])

                # ---- table load: 8 group reads of the same DRAM region
                tabt = tabp.tile([P, SLICE], dt.int32, tag="tabt")
                for g in range(8):
                    nc.sync.dma_start(tabt[16 * g:16 * (g + 1), :], tblr[lvl])

                # ---- whole-level hash into LO/HI (int16 idx streams)
                lo = lhp.tile([P, SPP], dt.int16, tag="lo")
                hi = lhp.tile([P, SPP], dt.int16, tag="hi")
                for hc in range(NHC):
                    sl = slice(hc * HC, (hc + 1) * HC)
                    xt = hashio.tile([P, HC], dt.float32, tag="xt")
                    yt = hashio.tile([P, HC], dt.float32, tag="yt")
                    zt = hashio.tile([P, HC], dt.float32, tag="zt")
                    nc.sync.dma_start(xt[:], coordsd[0, :, sl])
                    nc.sync.dma_start(yt[:], coordsd[1, :, sl])
                    nc.sync.dma_start(zt[:], coordsd[2, :, sl])

                    # g = floor(fl(x*R)) exactly: ACT round + is_lt fix
                    gs = []
                    rf = hashp.tile([P, HC], dt.float32, tag="rf")
                    cc = hashp.tile([P, HC], dt.int32, tag="cc")
                    for name, t in (("rx", xt), ("ry", yt), ("rz", zt)):
                        ri = hashp.tile([P, HC], dt.int32, tag=name)
                        nc.scalar.activation(ri[:], t[:], ActFn.Copy, scale=R)
                        nc.scalar.activation(rf[:], ri[:], ActFn.Copy)
                        nc.vector.scalar_tensor_tensor(
                            cc[:], t[:], R, rf[:], AluOp.mult, AluOp.is_lt)
                        nc.vector.tensor_tensor(ri[:], ri[:], cc[:],
                                                AluOp.subtract)
                        gs.append(ri)
                    gx, gy, gz = gs

                    # p = (g*K') mod 2^19 via Khi/Klo split (f32-exact)
                    at = hashp.tile([P, HC], dt.int32, tag="at")
                    bt = hashp.tile([P, HC], dt.int32, tag="bt")
                    nc.vector.tensor_scalar(at[:], gy[:], KH1, None,
                                            AluOp.mult)
                    nc.vector.tensor_scalar(at[:], at[:], 0xFFF, None,
                                            AluOp.bitwise_and)
                    nc.scalar.activation(bt[:], gy[:], ActFn.Copy,
                                         scale=float(KL1))
                    nc.vector.scalar_tensor_tensor(at[:], at[:], 128, bt[:],
                                                   AluOp.mult, AluOp.add)
                    dt_ = hashp.tile([P, HC], dt.int32, tag="dt")
                    nc.vector.tensor_scalar(dt_[:], gz[:], KH2, None,
                                            AluOp.mult)
                    nc.vector.tensor_scalar(dt_[:], dt_[:], 0xFFF, None,
                                            AluOp.bitwise_and)
                    nc.scalar.activation(bt[:], gz[:], ActFn.Copy,
                                         scale=float(KL2))
                    nc.vector.scalar_tensor_tensor(dt_[:], dt_[:], 128, bt[:],
                                                   AluOp.mult, AluOp.add)

                    # h = gx ^ py ^ pz; lo = h & 0x7FFF; hi = (h>>15) & 0xF
                    nc.vector.tensor_tensor(gx[:], gx[:], at[:],
                                            AluOp.bitwise_xor)
                    nc.vector.tensor_tensor(gx[:], gx[:], dt_[:],
                                            AluOp.bitwise_xor)
                    nc.vector.tensor_scalar(cc[:], gx[:], 0x7FFF, None,
                                            AluOp.bitwise_and)
                    nc.scalar.activation(lo[:, sl], cc[:], ActFn.Copy)
                    nc.vector.tensor_scalar(cc[:], gx[:], 15, 0xF,
                                            AluOp.logical_shift_right,
                                            AluOp.bitwise_and)
                    nc.scalar.activation(hi[:, sl], cc[:], ActFn.Copy)

                # ---- gather / select / group-sum / evac
                for c in range(NGC):
                    isl = slice(c * ICH, (c + 1) * ICH)
                    cand = gbufp.tile([P, GC], dt.int32, tag="cand")
                    nc.gpsimd.ap_gather(cand[:], tabt[:], lo[:, isl],
                                        channels=P, num_elems=SLICE, d=1,
                                        num_idxs=GC)
                    mask = gbufp.tile([P, GC], dt.int32, tag="mask")
                    nc.gpsimd.ap_gather(mask[:], ind[:], hi[:, isl],
                                        channels=P, num_elems=16, d=1,
                                        num_idxs=GC)
                    nc.vector.tensor_tensor(cand[:], cand[:], mask[:],
                                            AluOp.bitwise_and)

                    selb = cand[:].bitcast(dt.bfloat16)  # [P, 2*GC]
                    ps = psp.tile([8, 2 * GC], dt.float32, space="PSUM",
                                  tag="ps")
                    for q in range(2 * GC // 512):
                        qs = slice(512 * q, 512 * (q + 1))
                        nc.tensor.matmul(ps[:, qs], b16[:], selb[:, qs],
                                         start=True, stop=True)

                    if c % 2 == 0:
                        ev = evp.tile([8, 4 * GC], dt.bfloat16, tag="ev")
                    esl = slice((c % 2) * 2 * GC, (c % 2 + 1) * 2 * GC)
                    if c % EVAC_DVE_MOD == EVAC_DVE_MOD - 1:
                        nc.vector.tensor_copy(ev[:, esl], ps[:])
                    else:
                        nc.scalar.activation(ev[:, esl], ps[:], ActFn.Copy)
                    if c % 2 == 1:
                        nc.sync.dma_start(outd[lvl, c // 2], ev[:])

    nc.compile()
    return nc


def _prep_tables(tables):
    """[L, T, F] f32 -> packed bf16-pair int32 [L, 16, SLICE]."""
    tb = tables.astype(ml_dtypes.bfloat16).view(np.uint16)  # [L, T, 2]
    pk = tb[..., 0].astype(np.uint32) | (tb[..., 1].astype(np.uint32) << 16)
    return pk.view(np.int32).reshape(N_LEVELS, 16, SLICE)


def kernel(coords, tables):
    global LAST_EXEC_SECONDS, LAST_RESULT
    import os
    import time
    t_start = time.time()
    coords = np.asarray(coords, dtype=np.float32)
    tables = np.asarray(tables, dtype=np.float32)

    tblr = _prep_tables(tables)
    b16 = np.zeros((P, 8), ml_dtypes.bfloat16)
    for g in range(8):
        b16[16 * g:16 * (g + 1), g] = 1
    ind = np.zeros((P, 16), np.int32)
    for p in range(P):
        ind[p, p % 16] = -1
    LAST_TIMES["prep"] = time.time() - t_start

    t0 = time.time()
    nc = build_nc()
    LAST_TIMES["build"] = time.time() - t0

    t0 = time.time()
    in_maps = []
    for c in range(N_CORES):
        csl = coords[c * BC:(c + 1) * BC]  # [BC, 3]
        c3 = np.ascontiguousarray(csl.T.reshape(3, P, SPP))
        in_maps.append({"coords3": c3, "tblr": tblr, "b16": b16, "ind": ind})
    LAST_TIMES["inmaps"] = time.time() - t0

    t0 = time.time()
    res = run_bass_kernel_spmd(nc, in_maps, core_ids=list(range(N_CORES)),
                               tmpdir=os.environ.get("BASS_TMPDIR"))
    LAST_EXEC_SECONDS = time.time() - t0
    LAST_TIMES["spmd"] = LAST_EXEC_SECONDS
    LAST_RESULT = res

    t0 = time.time()
    out = np.empty((BATCH, N_LEVELS * N_FEATS), np.float32)
    for c in range(N_CORES):
        oc = res.results[c]["out"]  # [L, 16, 8, 4096] bf16-as-stored
        oc = np.asarray(oc).view(ml_dtypes.bfloat16) if oc.dtype != ml_dtypes.bfloat16 else oc
        # [L, NGC/2, 8, 2(half), ICH(t), 16(q), 2(f)]
        oc = oc.reshape(N_LEVELS, NGC // 2, 8, 2, ICH, 16, N_FEATS)
        # b = (16g+q)*SPP + (2*cc+half)*ICH + t ; feature = 2l + f
        oc = oc.transpose(2, 5, 1, 3, 4, 0, 6)  # g, q, cc, half, t, L, f
        out[c * BC:(c + 1) * BC] = oc.reshape(
            BC, N_LEVELS * N_FEATS).astype(np.float32)
    LAST_TIMES["unscramble"] = time.time() - t0
    return out


# revision 10
# speedup vs baseline: 1.0067x; 1.0067x over previous
"""MultiResolutionHashEncoding Trainium2 kernel (v1 redesign).

Per core (batch sharded 8 ways, 262144 elements = 128 partitions x 2048):
  - Tables stored as packed bf16 pairs in int32, one 32768-entry slice per
    partition (slice p%16); all 8 gpsimd-core groups read the same compact
    DRAM copy (replication happens in the 8 HBM->SBUF loads).
  - Whole-level hash on DVE+ACT: floor via ACT round + fused is_lt
    correction, prime multiplies split Khi/Klo to stay f32-exact,
    19-bit arithmetic via fused two-scalar tensor_scalar ops.
  - ap_gather (d=1 int32) fetches candidates on all 16 partitions of each
    group; a second tiny gather produces {-1,0} masks; DVE AND selects.
  - PE sums each 16-partition group via block-diagonal ones (bf16),
    PSUM evacuated to bf16 by ACT (mostly) and DVE (some, for balance).
  - Output bf16 [L, 16, 8, 4096] per core, unscrambled + f32-cast on host.
"""

import numpy as np
import ml_dtypes

import concourse.tile_utils as tile_utils

tile_utils.max_sbuf_usage = 206 * 1024

import concourse.bacc as bacc
import concourse.tile as tile
import concourse.mybir as mybir
from concourse.bass_utils import run_bass_kernel_spmd

AluOp = mybir.AluOpType
ActFn = mybir.ActivationFunctionType
dt = mybir.dt

N_LEVELS = 16
N_FEATS = 2
TABLE_SIZE = 524288  # 2**19
RESOLUTIONS = [16, 23, 32, 45, 64, 91, 128, 181, 256, 362, 512, 724, 1024,
               1448, 2048, 2896]
BATCH = 2_097_152
N_CORES = 8

P = 128
BC = BATCH // N_CORES          # 262144 elements per core
SPP = BC // P                  # 2048 elements per partition
SLICE = 32768                  # table entries per partition slice
MASK19 = 0x7FFFF

K1 = 2654435761 & MASK19       # 489905
K2 = 805459861 & MASK19        # 95765
KH1, KL1 = K1 >> 7, K1 & 127   # 3827, 49
KH2, KL2 = K2 >> 7, K2 & 127   # 748, 21

HC = 512                       # hash sub-chunk columns
NHC = SPP // HC                # 4 hash sub-chunks per level
GC = 1024                      # gather-chunk indices per gpsimd core
NGC = (16 * SPP) // GC         # 32 gather chunks per level
ICH = GC // 16                 # 64 idx columns per partition per chunk
EVAC_DVE_MOD = 8               # chunk c evacs on DVE when c % MOD == MOD-1

LAST_EXEC_SECONDS = None
LAST_RESULT = None
LAST_TIMES = {}


def build_nc():
    nc = bacc.Bacc(None, target_bir_lowering=False)

    coordsd = nc.dram_tensor("coords3", [3, P, SPP], dt.float32,
                             kind="ExternalInput")
    tblr = nc.dram_tensor("tblr", [N_LEVELS, 16, SLICE, N_FEATS], dt.bfloat16,
                          kind="ExternalInput")
    b16d = nc.dram_tensor("b16", [P, 8], dt.bfloat16, kind="ExternalInput")
    indd = nc.dram_tensor("ind", [P, 16, N_FEATS], dt.bfloat16,
                          kind="ExternalInput")
    outd = nc.dram_tensor("out", [N_LEVELS, NGC // 2, 8, 2 * GC * N_FEATS],
                          dt.bfloat16, kind="ExternalOutput")

    with tile.TileContext(nc) as tc:
        with (
            tc.tile_pool(name="constp", bufs=1) as constp,
            tc.tile_pool(name="tabp", bufs=1) as tabp,
            tc.tile_pool(name="hashio", bufs=2) as hashio,
            tc.tile_pool(name="hashp", bufs=1) as hashp,
            tc.tile_pool(name="lhp", bufs=2) as lhp,
            tc.tile_pool(name="gbufp", bufs=2) as gbufp,
            tc.tile_pool(name="evp", bufs=2) as evp,
            tc.tile_pool(name="psp", bufs=2, space="PSUM") as psp,
        ):
            b16 = constp.tile([P, 8], dt.bfloat16, tag="b16")
            nc.sync.dma_start(b16[:], b16d[:])
            ind = constp.tile([P, 16, N_FEATS], dt.bfloat16, tag="ind")
            nc.sync.dma_start(ind[:], indd[:])

            for lvl in range(N_LEVELS):
                R = float(RESOLUTIONS[lvl])

                # ---- table load: 8 group reads of the same DRAM region,
                # phase-staggered so concurrent streams hit distinct HBM rows
                tabt = tabp.tile([P, SLICE, N_FEATS], dt.bfloat16, tag="tabt")
                for g in range(8):
                    gsl = slice(16 * g, 16 * (g + 1))
                    off = g * (SLICE // 8)
                    nc.sync.dma_start(tabt[gsl, off:, :],
                                      tblr[lvl, :, off:, :])
                    if off:
                        nc.sync.dma_start(tabt[gsl, :off, :],
                                          tblr[lvl, :, :off, :])

                # ---- whole-level hash into LO/HI (int16 idx streams)
                lo = lhp.tile([P, SPP], dt.int16, tag="lo")
                hi = lhp.tile([P, SPP], dt.int16, tag="hi")
                for hc in range(NHC):
                    sl = slice(hc * HC, (hc + 1) * HC)
                    xt = hashio.tile([P, HC], dt.float32, tag="xt")
                    yt = hashio.tile([P, HC], dt.float32, tag="yt")
                    zt = hashio.tile([P, HC], dt.float32, tag="zt")
                    nc.sync.dma_start(xt[:], coordsd[0, :, sl])
                    nc.sync.dma_start(yt[:], coordsd[1, :, sl])
                    nc.sync.dma_start(zt[:], coordsd[2, :, sl])

                    # g = floor(fl(x*R)) exactly: ACT round + is_lt fix
                    gs = []
                    rf = hashp.tile([P, HC], dt.float32, tag="rf")
                    cc = hashp.tile([P, HC], dt.int32, tag="cc")
                    for name, t in (("rx", xt), ("ry", yt), ("rz", zt)):
                        ri = hashp.tile([P, HC], dt.int32, tag=name)
                        nc.scalar.activation(ri[:], t[:], ActFn.Copy, scale=R)
                        nc.scalar.activation(rf[:], ri[:], ActFn.Copy)
                        nc.vector.scalar_tensor_tensor(
                            cc[:], t[:], R, rf[:], AluOp.mult, AluOp.is_lt)
                        nc.vector.tensor_tensor(ri[:], ri[:], cc[:],
                                                AluOp.subtract)
                        gs.append(ri)
                    gx, gy, gz = gs

                    # p = (g*K') mod 2^19 via Khi/Klo split (f32-exact)
                    at = hashp.tile([P, HC], dt.int32, tag="at")
                    bt = hashp.tile([P, HC], dt.int32, tag="bt")
                    nc.vector.tensor_scalar(at[:], gy[:], KH1, None,
                                            AluOp.mult)
                    nc.vector.tensor_scalar(at[:], at[:], 0xFFF, None,
                                            AluOp.bitwise_and)
                    nc.scalar.activation(bt[:], gy[:], ActFn.Copy,
                                         scale=float(KL1))
                    nc.vector.scalar_tensor_tensor(at[:], at[:], 128, bt[:],
                                                   AluOp.mult, AluOp.add)
                    dt_ = hashp.tile([P, HC], dt.int32, tag="dt")
                    nc.vector.tensor_scalar(dt_[:], gz[:], KH2, None,
                                            AluOp.mult)
                    nc.vector.tensor_scalar(dt_[:], dt_[:], 0xFFF, None,
                                            AluOp.bitwise_and)
                    nc.scalar.activation(bt[:], gz[:], ActFn.Copy,
                                         scale=float(KL2))
                    nc.vector.scalar_tensor_tensor(dt_[:], dt_[:], 128, bt[:],
                                                   AluOp.mult, AluOp.add)

                    # h = gx ^ py ^ pz; lo = h & 0x7FFF; hi = (h>>15) & 0xF
                    nc.vector.tensor_tensor(gx[:], gx[:], at[:],
                                            AluOp.bitwise_xor)
                    nc.vector.tensor_tensor(gx[:], gx[:], dt_[:],
                                            AluOp.bitwise_xor)
                    nc.vector.tensor_scalar(cc[:], gx[:], 0x7FFF, None,
                                            AluOp.bitwise_and)
                    nc.scalar.activation(lo[:, sl], cc[:], ActFn.Copy)
                    nc.vector.tensor_scalar(cc[:], gx[:], 15, 0xF,
                                            AluOp.logical_shift_right,
                                            AluOp.bitwise_and)
                    nc.scalar.activation(hi[:, sl], cc[:], ActFn.Copy)

                # ---- gather / select / group-sum / evac
                for c in range(NGC):
                    isl = slice(c * ICH, (c + 1) * ICH)
                    cand = gbufp.tile([P, GC, N_FEATS], dt.bfloat16,
                                      tag="cand")
                    nc.gpsimd.ap_gather(cand[:], tabt[:], lo[:, isl],
                                        channels=P, num_elems=SLICE,
                                        d=N_FEATS, num_idxs=GC)
                    mask = gbufp.tile([P, GC, N_FEATS], dt.bfloat16,
                                      tag="mask")
                    nc.gpsimd.ap_gather(mask[:], ind[:], hi[:, isl],
                                        channels=P, num_elems=16, d=N_FEATS,
                                        num_idxs=GC)
                    selb = cand[:].rearrange("p n f -> p (n f)")
                    mflt = mask[:].rearrange("p n f -> p (n f)")
                    nc.vector.tensor_tensor(selb, selb, mflt, AluOp.mult)
                    ps = psp.tile([8, 2 * GC], dt.float32, space="PSUM",
                                  tag="ps")
                    for q in range(2 * GC // 512):
                        qs = slice(512 * q, 512 * (q + 1))
                        nc.tensor.matmul(ps[:, qs], b16[:], selb[:, qs],
                                         start=True, stop=True)

                    if c % 2 == 0:
                        ev = evp.tile([8, 4 * GC], dt.bfloat16, tag="ev")
                    esl = slice((c % 2) * 2 * GC, (c % 2 + 1) * 2 * GC)
                    if c % EVAC_DVE_MOD == EVAC_DVE_MOD - 1:
                        nc.vector.tensor_copy(ev[:, esl], ps[:])
                    else:
                        nc.scalar.activation(ev[:, esl], ps[:], ActFn.Copy)
                    if c % 2 == 1:
                        nc.sync.dma_start(outd[lvl, c // 2], ev[:])

    nc.compile()
    return nc


def _prep_tables(tables):
    """[L, T, F] f32 -> bf16 [L, 16, SLICE, F] (slice-major)."""
    tb = tables.astype(ml_dtypes.bfloat16)
    return np.ascontiguousarray(tb.reshape(N_LEVELS, 16, SLICE, N_FEATS))


def kernel(coords, tables):
    global LAST_EXEC_SECONDS, LAST_RESULT
    import os
    import time
    t_start = time.time()
    coords = np.asarray(coords, dtype=np.float32)
    tables = np.asarray(tables, dtype=np.float32)

    tblr = _prep_tables(tables)
    b16 = np.zeros((P, 8), ml_dtypes.bfloat16)
    for g in range(8):
        b16[16 * g:16 * (g + 1), g] = 1
    ind = np.zeros((P, 16, N_FEATS), ml_dtypes.bfloat16)
    for p in range(P):
        ind[p, p % 16, :] = 1
    LAST_TIMES["prep"] = time.time() - t_start

    t0 = time.time()
    nc = build_nc()
    LAST_TIMES["build"] = time.time() - t0

    t0 = time.time()
    in_maps = []
    for c in range(N_CORES):
        csl = coords[c * BC:(c + 1) * BC]  # [BC, 3]
        c3 = np.ascontiguousarray(csl.T.reshape(3, P, SPP))
        in_maps.append({"coords3": c3, "tblr": tblr, "b16": b16, "ind": ind})
    LAST_TIMES["inmaps"] = time.time() - t0

    t0 = time.time()
    res = run_bass_kernel_spmd(nc, in_maps, core_ids=list(range(N_CORES)),
                               tmpdir=os.environ.get("BASS_TMPDIR"))
    LAST_EXEC_SECONDS = time.time() - t0
    LAST_TIMES["spmd"] = LAST_EXEC_SECONDS
    LAST_RESULT = res

    t0 = time.time()
    out = np.empty((BATCH, N_LEVELS * N_FEATS), np.float32)
    for c in range(N_CORES):
        oc = res.results[c]["out"]  # [L, 16, 8, 4096] bf16-as-stored
        oc = np.asarray(oc).view(ml_dtypes.bfloat16) if oc.dtype != ml_dtypes.bfloat16 else oc
        # [L, NGC/2, 8, 2(half), ICH(t), 16(q), 2(f)]
        oc = oc.reshape(N_LEVELS, NGC // 2, 8, 2, ICH, 16, N_FEATS)
        # b = (16g+q)*SPP + (2*cc+half)*ICH + t ; feature = 2l + f
        oc = oc.transpose(2, 5, 1, 3, 4, 0, 6)  # g, q, cc, half, t, L, f
        out[c * BC:(c + 1) * BC] = oc.reshape(
            BC, N_LEVELS * N_FEATS).astype(np.float32)
    LAST_TIMES["unscramble"] = time.time() - t0
    return out


# revision 13
# speedup vs baseline: 1.0308x; 1.0239x over previous
"""MultiResolutionHashEncoding Trainium2 kernel (v1 redesign).

Per core (batch sharded 8 ways, 262144 elements = 128 partitions x 2048):
  - Tables stored as packed bf16 pairs in int32, one 32768-entry slice per
    partition (slice p%16); all 8 gpsimd-core groups read the same compact
    DRAM copy (replication happens in the 8 HBM->SBUF loads).
  - Whole-level hash on DVE+ACT: floor via ACT round + fused is_lt
    correction, prime multiplies split Khi/Klo to stay f32-exact,
    19-bit arithmetic via fused two-scalar tensor_scalar ops.
  - ap_gather (d=1 int32) fetches candidates on all 16 partitions of each
    group; a second tiny gather produces {-1,0} masks; DVE AND selects.
  - PE sums each 16-partition group via block-diagonal ones (bf16),
    PSUM evacuated to bf16 by ACT (mostly) and DVE (some, for balance).
  - Output bf16 [L, 16, 8, 4096] per core, unscrambled + f32-cast on host.
"""

import numpy as np
import ml_dtypes

import concourse.tile_utils as tile_utils

tile_utils.max_sbuf_usage = 206 * 1024

import concourse.bacc as bacc
import concourse.tile as tile
import concourse.mybir as mybir
from concourse.bass_utils import run_bass_kernel_spmd

AluOp = mybir.AluOpType
ActFn = mybir.ActivationFunctionType
dt = mybir.dt

N_LEVELS = 16
N_FEATS = 2
TABLE_SIZE = 524288  # 2**19
RESOLUTIONS = [16, 23, 32, 45, 64, 91, 128, 181, 256, 362, 512, 724, 1024,
               1448, 2048, 2896]
BATCH = 2_097_152
N_CORES = 8

P = 128
BC = BATCH // N_CORES          # 262144 elements per core
SPP = BC // P                  # 2048 elements per partition
SLICE = 32768                  # table entries per partition slice
MASK19 = 0x7FFFF

K1 = 2654435761 & MASK19       # 489905
K2 = 805459861 & MASK19        # 95765
KH1, KL1 = K1 >> 7, K1 & 127   # 3827, 49
KH2, KL2 = K2 >> 7, K2 & 127   # 748, 21

HC = 256                       # hash sub-chunk columns
NHC = SPP // HC                # 4 hash sub-chunks per level
GC = 2048                      # gather-chunk indices per gpsimd core
NGC = (16 * SPP) // GC         # 32 gather chunks per level
ICH = GC // 16                 # 64 idx columns per partition per chunk
EVAC_DVE_MOD = 8               # chunk c evacs on DVE when c % MOD == MOD-1

LAST_EXEC_SECONDS = None
LAST_RESULT = None
LAST_TIMES = {}


def build_nc():
    nc = bacc.Bacc(None, target_bir_lowering=False)

    coordsd = nc.dram_tensor("coords3", [3, P, SPP], dt.float32,
                             kind="ExternalInput")
    tblr = nc.dram_tensor("tblr", [N_LEVELS, 16, SLICE, N_FEATS], dt.bfloat16,
                          kind="ExternalInput")
    b16d = nc.dram_tensor("b16", [P, 8], dt.bfloat16, kind="ExternalInput")
    indd = nc.dram_tensor("ind", [P, 16, N_FEATS], dt.bfloat16,
                          kind="ExternalInput")
    outd = nc.dram_tensor("out", [N_LEVELS, NGC, 8, GC * N_FEATS],
                          dt.bfloat16, kind="ExternalOutput")

    with tile.TileContext(nc) as tc:
        with (
            tc.tile_pool(name="constp", bufs=1) as constp,
            tc.tile_pool(name="tabp", bufs=1) as tabp,
            tc.tile_pool(name="hashio", bufs=1) as hashio,
            tc.tile_pool(name="hashp", bufs=1) as hashp,
            tc.tile_pool(name="lhp", bufs=2) as lhp,
            tc.tile_pool(name="gbufp", bufs=2) as gbufp,
            tc.tile_pool(name="evp", bufs=2) as evp,
            tc.tile_pool(name="psp", bufs=1, space="PSUM") as psp,
        ):
            b16 = constp.tile([P, 8], dt.bfloat16, tag="b16")
            nc.sync.dma_start(b16[:], b16d[:])
            ind = constp.tile([P, 16, N_FEATS], dt.bfloat16, tag="ind")
            nc.sync.dma_start(ind[:], indd[:])

            for lvl in range(N_LEVELS):
                R = float(RESOLUTIONS[lvl])

                # ---- table load: 8 group reads of the same DRAM region,
                # phase-staggered so concurrent streams hit distinct HBM rows
                tabt = tabp.tile([P, SLICE, N_FEATS], dt.bfloat16, tag="tabt")
                for g in range(8):
                    gsl = slice(16 * g, 16 * (g + 1))
                    off = g * (SLICE // 8)
                    nc.sync.dma_start(tabt[gsl, off:, :],
                                      tblr[lvl, :, off:, :])
                    if off:
                        nc.sync.dma_start(tabt[gsl, :off, :],
                                          tblr[lvl, :, :off, :])

                # ---- whole-level hash into LO/HI (int16 idx streams)
                lo = lhp.tile([P, SPP], dt.int16, tag="lo")
                hi = lhp.tile([P, SPP], dt.int16, tag="hi")
                for hc in range(NHC):
                    sl = slice(hc * HC, (hc + 1) * HC)
                    xt = hashio.tile([P, HC], dt.float32, tag="xt")
                    yt = hashio.tile([P, HC], dt.float32, tag="yt")
                    zt = hashio.tile([P, HC], dt.float32, tag="zt")
                    nc.sync.dma_start(xt[:], coordsd[0, :, sl])
                    nc.sync.dma_start(yt[:], coordsd[1, :, sl])
                    nc.sync.dma_start(zt[:], coordsd[2, :, sl])

                    # g = floor(fl(x*R)) exactly: ACT round + is_lt fix
                    gs = []
                    rf = hashp.tile([P, HC], dt.float32, tag="rf")
                    cc = hashp.tile([P, HC], dt.int32, tag="cc")
                    for name, t in (("rx", xt), ("ry", yt), ("rz", zt)):
                        ri = hashp.tile([P, HC], dt.int32, tag=name)
                        nc.scalar.activation(ri[:], t[:], ActFn.Copy, scale=R)
                        nc.scalar.activation(rf[:], ri[:], ActFn.Copy)
                        nc.vector.scalar_tensor_tensor(
                            cc[:], t[:], R, rf[:], AluOp.mult, AluOp.is_lt)
                        nc.vector.tensor_tensor(ri[:], ri[:], cc[:],
                                                AluOp.subtract)
                        gs.append(ri)
                    gx, gy, gz = gs

                    # p = (g*K') mod 2^19 via Khi/Klo split (f32-exact)
                    at = hashp.tile([P, HC], dt.int32, tag="at")
                    bt = hashp.tile([P, HC], dt.int32, tag="bt")
                    nc.vector.tensor_scalar(at[:], gy[:], KH1, None,
                                            AluOp.mult)
                    nc.vector.tensor_scalar(at[:], at[:], 0xFFF, None,
                                            AluOp.bitwise_and)
                    nc.scalar.activation(bt[:], gy[:], ActFn.Copy,
                                         scale=float(KL1))
                    nc.vector.scalar_tensor_tensor(at[:], at[:], 128, bt[:],
                                                   AluOp.mult, AluOp.add)
                    dt_ = hashp.tile([P, HC], dt.int32, tag="dt")
                    nc.vector.tensor_scalar(dt_[:], gz[:], KH2, None,
                                            AluOp.mult)
                    nc.vector.tensor_scalar(dt_[:], dt_[:], 0xFFF, None,
                                            AluOp.bitwise_and)
                    nc.scalar.activation(bt[:], gz[:], ActFn.Copy,
                                         scale=float(KL2))
                    nc.vector.scalar_tensor_tensor(dt_[:], dt_[:], 128, bt[:],
                                                   AluOp.mult, AluOp.add)

                    # h = gx ^ py ^ pz; lo = h & 0x7FFF; hi = (h>>15) & 0xF
                    nc.vector.tensor_tensor(gx[:], gx[:], at[:],
                                            AluOp.bitwise_xor)
                    nc.vector.tensor_tensor(gx[:], gx[:], dt_[:],
                                            AluOp.bitwise_xor)
                    nc.vector.tensor_scalar(cc[:], gx[:], 0x7FFF, None,
                                            AluOp.bitwise_and)
                    nc.scalar.activation(lo[:, sl], cc[:], ActFn.Copy)
                    nc.vector.tensor_scalar(cc[:], gx[:], 15, 0xF,
                                            AluOp.logical_shift_right,
                                            AluOp.bitwise_and)
                    nc.scalar.activation(hi[:, sl], cc[:], ActFn.Copy)

                # ---- gather / select / group-sum / evac
                for c in range(NGC):
                    isl = slice(c * ICH, (c + 1) * ICH)
                    cand = gbufp.tile([P, GC, N_FEATS], dt.bfloat16,
                                      tag="cand")
                    nc.gpsimd.ap_gather(cand[:], tabt[:], lo[:, isl],
                                        channels=P, num_elems=SLICE,
                                        d=N_FEATS, num_idxs=GC)
                    mask = gbufp.tile([P, GC, N_FEATS], dt.bfloat16,
                                      tag="mask")
                    nc.gpsimd.ap_gather(mask[:], ind[:], hi[:, isl],
                                        channels=P, num_elems=16, d=N_FEATS,
                                        num_idxs=GC)
                    selb = cand[:].rearrange("p n f -> p (n f)")
                    mflt = mask[:].rearrange("p n f -> p (n f)")
                    nc.vector.tensor_tensor(selb, selb, mflt, AluOp.mult)
                    ps = psp.tile([8, 2 * GC], dt.float32, space="PSUM",
                                  tag="ps")
                    for q in range(2 * GC // 512):
                        qs = slice(512 * q, 512 * (q + 1))
                        nc.tensor.matmul(ps[:, qs], b16[:], selb[:, qs],
                                         start=True, stop=True)

                    ev = evp.tile([8, 2 * GC], dt.bfloat16, tag="ev")
                    if c % EVAC_DVE_MOD == EVAC_DVE_MOD - 1:
                        nc.vector.tensor_copy(ev[:], ps[:])
                    else:
                        nc.scalar.activation(ev[:], ps[:], ActFn.Copy)
                    nc.sync.dma_start(outd[lvl, c], ev[:])

    nc.compile()
    return nc


def _prep_tables(tables):
    """[L, T, F] f32 -> bf16 [L, 16, SLICE, F] (slice-major)."""
    tb = tables.astype(ml_dtypes.bfloat16)
    return np.ascontiguousarray(tb.reshape(N_LEVELS, 16, SLICE, N_FEATS))


def kernel(coords, tables):
    global LAST_EXEC_SECONDS, LAST_RESULT
    import os
    import time
    t_start = time.time()
    coords = np.asarray(coords, dtype=np.float32)
    tables = np.asarray(tables, dtype=np.float32)

    tblr = _prep_tables(tables)
    b16 = np.zeros((P, 8), ml_dtypes.bfloat16)
    for g in range(8):
        b16[16 * g:16 * (g + 1), g] = 1
    ind = np.zeros((P, 16, N_FEATS), ml_dtypes.bfloat16)
    for p in range(P):
        ind[p, p % 16, :] = 1
    LAST_TIMES["prep"] = time.time() - t_start

    t0 = time.time()
    nc = build_nc()
    LAST_TIMES["build"] = time.time() - t0

    t0 = time.time()
    in_maps = []
    for c in range(N_CORES):
        csl = coords[c * BC:(c + 1) * BC]  # [BC, 3]
        c3 = np.ascontiguousarray(csl.T.reshape(3, P, SPP))
        in_maps.append({"coords3": c3, "tblr": tblr, "b16": b16, "ind": ind})
    LAST_TIMES["inmaps"] = time.time() - t0

    t0 = time.time()
    res = run_bass_kernel_spmd(nc, in_maps, core_ids=list(range(N_CORES)),
                               tmpdir=os.environ.get("BASS_TMPDIR"))
    LAST_EXEC_SECONDS = time.time() - t0
    LAST_TIMES["spmd"] = LAST_EXEC_SECONDS
    LAST_RESULT = res

    t0 = time.time()
    out = np.empty((BATCH, N_LEVELS * N_FEATS), np.float32)
    for c in range(N_CORES):
        oc = res.results[c]["out"]  # [L, 16, 8, 4096] bf16-as-stored
        oc = np.asarray(oc).view(ml_dtypes.bfloat16) if oc.dtype != ml_dtypes.bfloat16 else oc
        # [L, NGC, 8, ICH(t), 16(q), 2(f)]
        oc = oc.reshape(N_LEVELS, NGC, 8, ICH, 16, N_FEATS)
        # b = (16g+q)*SPP + c*ICH + t ; feature = 2l + f
        oc = oc.transpose(2, 4, 1, 3, 0, 5)  # g, q, c, t, L, f
        out[c * BC:(c + 1) * BC] = oc.reshape(
            BC, N_LEVELS * N_FEATS).astype(np.float32)
    LAST_TIMES["unscramble"] = time.time() - t0
    return out
